# revision 32
# baseline (speedup 1.0000x reference)
"""Bass/Trainium2 kernel for nn_LocalAggregator (GNN message passing).

Math per batch b (hidden [64,128], adj [64,64] in {0..4}, a [4,128]):
    e_k[i,j] = leakyrelu_{0.2}( sum_d hidden[i,d]*hidden[j,d]*a[k,d] )
    alpha    = softmax_j( where(adj==k+1, e_k, -9e15) )
    out      = alpha @ hidden

Device strategy (8 cores, 64 batches/core, OCT = 8 batches/iter):
  - e_k is SYMMETRIC in (i,j): masking with the host-TRANSPOSED
    adjacency yields transposed attention weights directly.
  - w_all[d,(k,g',l,j)] = hT * a_k: HALF shipped from host (k=0,1),
    half computed on device (k=2 on DVE tensor_scalar, k=3 on ACT
    scaled-Copy, both with per-partition a_k scalars) -- trades 2.1MB
    of HBM traffic for ~1us/oct of idle engine time.  The DMA'd half
    lands directly in the same SBUF tile the computed half writes, so
    e-matmul rhs views span one tile.
  - The one-hot additive mask ships as fp8 {0,-192} bytes inside the
    bf16 input tensor (bitcast view on SBUF) and is ADDED into the
    e-PSUM by an fp8 identity matmul (start=True) before the
    e-matmuls accumulate.  The per-k select then becomes a single
    max-reduce; lrelu keeps masked entries <= -38 so exp ~ 0 there.
    ACT per gp: Prelu[128] + Exp[128] only.
  - out-matmuls run ONE OCT BEHIND the e-chain (software pipelining)
    and scratch filler matmuls bridge PE gaps: the PE only reaches
    full clock (2.4GHz) after ~3us of gap-free execution.
  - ones-column in hh makes the out-matmul emit the softmax
    denominator; normalization on HOST.
"""

import numpy as np
import ml_dtypes

from contextlib import ExitStack

import concourse.bass as bass
import concourse.tile as tile
from concourse import bacc, mybir
from concourse._compat import with_exitstack
from concourse.bass_utils import run_bass_kernel_spmd

BF16 = mybir.dt.bfloat16
FP16 = mybir.dt.float16
FP8 = mybir.dt.float8e4
F32 = mybir.dt.float32
ALU = mybir.AluOpType
ACTF = mybir.ActivationFunctionType

B, N, D, K = 512, 64, 128, 4
NCORES = 8
BPC = B // NCORES          # 64 batches per core
OCTS = BPC // 8            # 8 octs of 8 batches per core
HHW = 132                  # hidden cols + ones col + pad
MASK = -192.0              # additive mask; exact in fp8e4m3, exp() -> ~0
CWA = 512 + 528 + 512      # hT8 | hh8 | indm(fp8 as bf16) = 1552 cols
CWB = 1024                 # wall8 k=0,1 half (2048B rows)
OWC = 4 * 129              # out tile cols: (num 128 | denom) x (gp,t)


@with_exitstack
def _kernel_body(ctx, tc, cmba_d, wall_d, id_d, at_d, out_d):
    nc = tc.nc

    const_pool = ctx.enter_context(tc.tile_pool(name="const", bufs=1))
    ina_pool = ctx.enter_context(tc.tile_pool(name="inpa", bufs=6))
    wl_pool = ctx.enter_context(tc.tile_pool(name="wlp", bufs=4))
    work_pool = ctx.enter_context(tc.tile_pool(name="work", bufs=4))
    wal_pool = ctx.enter_context(tc.tile_pool(name="walp", bufs=6))
    psum_pool = ctx.enter_context(tc.tile_pool(name="psum", bufs=3, space="PSUM"))
    opsum_pool = ctx.enter_context(tc.tile_pool(name="opsum", bufs=2, space="PSUM"))
    scr_pool = ctx.enter_context(tc.tile_pool(name="scr", bufs=1, space="PSUM"))
    out_pool = ctx.enter_context(tc.tile_pool(name="outp", bufs=4))

    ident = const_pool.tile([128, 128], FP8, tag="ident")
    nc.sync.dma_start(out=ident[:], in_=id_d)
    aT = const_pool.tile([128, 4], F32, tag="aT")
    nc.sync.dma_start(out=aT[:], in_=at_d)

    # PE keep-warm: the tensor engine only reaches full clock after ~3us
    # of gap-free execution.  A scratch stream with no input deps bridges
    # the startup (DMA latency) and the per-oct gaps so the real matmuls
    # run at full rate.  Results are never read.
    fsrc = const_pool.tile([128, 512], FP8, tag="fsrc")
    nc.gpsimd.memset(fsrc[:], 0)
    scr = scr_pool.tile([128, 512], F32, tag="scr")

    def filler(n):
        for _ in range(n):
            nc.tensor.matmul(scr[:], lhsT=fsrc[:, 0:128], rhs=fsrc[:],
                             start=True, stop=True)

    filler(10)

    def out_block(g, wal, hh8, final=False):
        """out matmuls + evac + output DMA for oct g (runs 1 oct behind
        the e-chain so the PE never waits on the ACT/DVE chain).  The
        final oct evacuates per gp-half on both engines with HWDGE DMA
        halves to shorten the drain chain."""
        ops = opsum_pool.tile([128, 1024], F32, tag="ops")
        osb = out_pool.tile([128, OWC], BF16, tag="osb")
        for gp in range(2):
            for l in range(4):
                t, u = l // 2, l % 2
                nc.tensor.matmul(
                    ops[u * 64:(u + 1) * 64,
                        gp * 512 + t * HHW: gp * 512 + (t + 1) * HHW],
                    lhsT=wal[u * 64:(u + 1) * 64,
                             gp * 128 + t * 64: gp * 128 + (t + 1) * 64],
                    rhs=hh8[u * 64:(u + 1) * 64,
                            gp * 264 + t * HHW: gp * 264 + (t + 1) * HHW],
                    start=True, stop=True,
                    tile_position=(u * 64, u * 64))
            if final:
                # evac this gp-half as soon as its 4 matmuls finish
                srcg = (ops[:, gp * 512:gp * 512 + 264]
                        .rearrange("p (t c) -> p t c", t=2)[:, :, 0:129])
                dstg = (osb[:, gp * 258:(gp + 1) * 258]
                        .rearrange("p (t c) -> p t c", t=2))
                if gp == 0:
                    nc.vector.tensor_copy(dstg, srcg)
                else:
                    nc.scalar.activation(dstg, srcg, ACTF.Copy)
                nc.sync.dma_start(out=out_d[g][:, gp * 258:(gp + 1) * 258],
                                  in_=osb[:, gp * 258:(gp + 1) * 258])
        if not final:
            # compact evac (num|den only), alternating DVE/ACT for balance
            src = ops[:].rearrange("p (g q) -> p g q", g=2)[:, :, 0:264]
            src = src.rearrange("p g (t c) -> p g t c", t=2)[:, :, :, 0:129]
            dst = osb[:].rearrange("p (g t c) -> p g t c", g=2, t=2)
            if g % 2 == 0:
                nc.vector.tensor_copy(dst, src)
            else:
                nc.scalar.activation(dst, src, ACTF.Copy)
            nc.gpsimd.dma_start(out=out_d[g], in_=osb[:])

    prev = None
    for g in range(OCTS):
        # oct loads: A = 0:512 hT8 [d,(g',l,i)] | 512:1040 hh8
        # [(u,j),(g',t,c)] | 1040:1552 fp8 mask bytes [(u,x),(k,g',t,y)]
        # B = wall k=0,1 [d,(k01,g',l,j)] -> first half of the wall tile
        cmba = ina_pool.tile([128, CWA], BF16, tag="cmba")
        nc.sync.dma_start(out=cmba[:], in_=cmba_d[g])
        wall = wl_pool.tile([128, 2048], BF16, tag="wall")
        nc.sync.dma_start(out=wall[:, 0:1024], in_=wall_d[g])

        # lagged out-block first: its inputs are long since ready, so the
        # PE stays busy while this oct's DMA lands
        if prev is not None:
            out_block(*prev)

        hT8 = cmba[:, 0:512]
        hh8 = cmba[:, 512:1040]
        im8v = cmba[:, 1040:CWA].bitcast(FP8).rearrange(
            "p (k g ty) -> p k g ty", k=4, g=2)

        # device-computed wall half: k=2 on DVE, k=3 on ACT (per-partition
        # a_k scalars; both engines have slack under the DMA window).  The
        # final oct's pair is priority-hoisted one iteration so it does not
        # queue behind oct-6's elementwise (its data arrives in time, so no
        # head-of-line blocking on the in-order queues).
        from contextlib import nullcontext
        hoist = tc.high_priority(offset=30) if g == OCTS - 1 else nullcontext()
        with hoist:
            nc.vector.tensor_scalar(wall[:, 1024:1536], hT8, aT[:, 2:3],
                                    None, ALU.mult)
            nc.scalar.activation(wall[:, 1536:2048], hT8, ACTF.Copy,
                                 scale=aT[:, 3:4])
        # wall cols are (k, g', l, j): k-stride 512
        wallv = wall[:].rearrange("p (k g l j) -> p k g l j",
                                  k=4, g=2, l=4)

        z = work_pool.tile([128, 256], FP16, tag="z")
        for gp in range(2):
            # ---- e4[(u,x), (k,t,y)] = e_k[x,y] + mask (1-bank tile) ----
            e4 = psum_pool.tile([128, 512], F32, tag="e4")
            e4v = e4[:].rearrange("p (k t y) -> p k t y", k=4, t=2)
            # mask lands first (identity matmul, start=True resets bank)
            nc.tensor.matmul(
                e4[:].rearrange("p (k ty) -> p k ty", k=4),
                lhsT=ident[:],
                rhs=im8v[:, :, gp, :],
                start=True, stop=False)
            for l in range(4):
                t, u = l // 2, l % 2
                nc.tensor.matmul(
                    e4v[u * 64:(u + 1) * 64, :, t, :],
                    lhsT=hT8[:, gp * 256 + l * 64: gp * 256 + (l + 1) * 64],
                    rhs=wallv[:, :, gp, l, :],
                    start=False, stop=True,
                    tile_position=(0, u * 64))

            # ---- per-k select: ONE max-reduce over the k axis (PSUM) ----
            nc.vector.tensor_reduce(
                z[:, gp * 128:(gp + 1) * 128],
                e4[:].rearrange("p (k ty) -> p ty k", k=4),
                mybir.AxisListType.X, ALU.max)

        if g < OCTS - 1:
            # ---- leakyrelu (masked stays <= -38) then exp, both gps ----
            pz = work_pool.tile([128, 256], FP16, tag="pz")
            nc.scalar.activation(pz[:], z[:], ACTF.Prelu, alpha=0.2)
            wal = wal_pool.tile([128, 256], BF16, tag="wal")
            nc.scalar.activation(wal[:], pz[:], ACTF.Exp)

            # bridge the oct-boundary PE gap to hold the clock at full
            # speed (more in the drain phase, where chain waits downclock)
            filler(1 if g < 5 else 3)
            prev = (g, wal, hh8)
        else:
            # ---- final oct: gp-granular drain.  Each half's exp ->
            # out-matmuls -> evac -> DMA proceeds without waiting for the
            # other half's reduce/activation chain.
            filler(3)
            ops = opsum_pool.tile([128, 1024], F32, tag="ops")
            osb = out_pool.tile([128, OWC], BF16, tag="osb")
            for gp in range(2):
                pzg = work_pool.tile([128, 128], FP16, tag="pzf")
                zg = z[:, gp * 128:(gp + 1) * 128]
                if gp == 0:
                    nc.scalar.activation(pzg[:], zg, ACTF.Prelu, alpha=0.2)
                else:
                    # lrelu = max(0.2*z, z) on DVE: keeps ACT's serial
                    # queue to [prelu0, exp0, exp1] in the drain
                    nc.vector.scalar_tensor_tensor(pzg[:], zg, 0.2, zg,
                                                   ALU.mult, ALU.max)
                walg = wal_pool.tile([128, 128], BF16, tag="walf")
                nc.scalar.activation(walg[:], pzg[:], ACTF.Exp)
                for l in range(4):
                    t, u = l // 2, l % 2
                    nc.tensor.matmul(
                        ops[u * 64:(u + 1) * 64,
                            gp * 512 + t * HHW: gp * 512 + (t + 1) * HHW],
                        lhsT=walg[u * 64:(u + 1) * 64, t * 64:(t + 1) * 64],
                        rhs=hh8[u * 64:(u + 1) * 64,
                                gp * 264 + t * HHW: gp * 264 + (t + 1) * HHW],
                        start=True, stop=True,
                        tile_position=(u * 64, u * 64))
                srcg = (ops[:, gp * 512:gp * 512 + 264]
                        .rearrange("p (t c) -> p t c", t=2)[:, :, 0:129])
                dstg = (osb[:, gp * 258:(gp + 1) * 258]
                        .rearrange("p (t c) -> p t c", t=2))
                if gp == 0:
                    nc.vector.tensor_copy(dstg, srcg)
                else:
                    nc.scalar.activation(dstg, srcg, ACTF.Copy)
                nc.sync.dma_start(out=out_d[g][:, gp * 258:(gp + 1) * 258],
                                  in_=osb[:, gp * 258:(gp + 1) * 258])
            prev = None

    if prev is not None:
        out_block(*prev, final=True)


def build_nc():
    nc = bacc.Bacc("TRN2", target_bir_lowering=False, debug=False)
    cmba_d = nc.dram_tensor("cmba", [OCTS, 128, CWA], BF16,
                            kind="ExternalInput").ap()
    wall_d = nc.dram_tensor("wall01", [OCTS, 128, CWB], BF16,
                            kind="ExternalInput").ap()
    id_d = nc.dram_tensor("ident", [128, 128], FP8,
                          kind="ExternalInput").ap()
    at_d = nc.dram_tensor("aT", [128, 4], F32,
                          kind="ExternalInput").ap()
    out_d = nc.dram_tensor("out", [OCTS, 128, OWC], BF16,
                           kind="ExternalOutput").ap()
    with tile.TileContext(nc) as tc:
        _kernel_body(tc, cmba_d, wall_d, id_d, at_d, out_d)
    nc.compile()
    return nc


def _octify(x):
    """[B//4, 128, W] -> [B//8, 128, 2*W] pairing consecutive quads."""
    q, p, w = x.shape
    return (x.reshape(q // 2, 2, p, w).transpose(0, 2, 1, 3)
            .reshape(q // 2, p, 2 * w))


def prep_inputs(hidden, adj, a):
    """Host-side packing: bf16/fp8 casts, fused transposed layouts."""
    bf = ml_dtypes.bfloat16
    f8 = ml_dtypes.float8_e4m3
    hidden = np.asarray(hidden, dtype=np.float32)
    adj = np.asarray(adj)
    a = np.asarray(a, dtype=np.float32)

    hb = hidden.astype(bf)                                   # [B, 64, 128]

    # hT_q[q, d, l*64+i] = hidden[4q+l, i, d]
    hTf = (hidden.transpose(0, 2, 1)
           .reshape(B // 4, 4, D, N)
           .transpose(0, 2, 1, 3)
           .reshape(B // 4, D, 4 * N))
    hT = hTf.astype(bf)

    # wall8[oct, d, k*512+g'*256+l*64+j] = hidden[oct*8+g'*4+l, j, d]*a[k,d]
    # shipped half from f32 hidden (single rounding; the device half pays
    # one extra bf16 rounding through hT)
    wallq = (hTf[:, None, :, :]
             * a[None, :, :, None]).astype(bf)               # [q, k, d, (l,j)]
    wall8 = (wallq.reshape(B // 8, 2, K, D, 4 * N)
             .transpose(0, 3, 2, 1, 4)
             .reshape(B // 8, D, 2 * K * 4 * N))             # [oct,d,(k,g,l,j)]
    wall01 = np.ascontiguousarray(wall8[:, :, 0:1024])

    # hh_q[q, u*64+j, t*HHW + c] : hidden rows + ones col for batch 4q+2t+u
    hh = np.zeros((B, N, HHW), dtype=bf)
    hh[:, :, 0:D] = hb
    hh[:, :, D] = bf(1.0)
    hhq = (hh.reshape(B // 4, 2, 2, N, HHW)
           .transpose(0, 2, 3, 1, 4)
           .reshape(B // 4, 2 * N, 2 * HHW))

    # indm[oct, u*64+x, k*256+g'*128+t*64+y] = 0 if adj[b][y,x]==k+1 else MASK
    # with b = oct*8 + g'*4 + t*2 + u; shipped as raw fp8 bytes inside cmba
    adjT = adj.transpose(0, 2, 1)                            # [b, x, y]
    mk = np.where(
        adjT[:, None, :, :] == np.arange(1, 5)[None, :, None, None],
        np.float32(0.0), np.float32(MASK)).astype(f8)        # [b, k, x, y]
    mk = mk.reshape(B // 8, 2, 2, 2, K, N, N)                # [o,g',t,u,k,x,y]
    indm = np.ascontiguousarray(
        mk.transpose(0, 3, 5, 4, 1, 2, 6).reshape(B // 8, 128, 1024))

    cmba = np.concatenate(
        [np.ascontiguousarray(_octify(hT)).view(np.uint8),
         np.ascontiguousarray(_octify(hhq)).view(np.uint8),
         indm.view(np.uint8)], axis=2).view(bf)
    cmba = np.ascontiguousarray(cmba)                        # [B//8, 128, CWA]

    ident = np.ascontiguousarray(np.eye(128, dtype=f8))
    aTh = np.ascontiguousarray(a.T.astype(np.float32))       # [128, 4]

    in_maps = []
    for c in range(NCORES):
        gsl = slice(c * OCTS, (c + 1) * OCTS)
        in_maps.append({"cmba": np.ascontiguousarray(cmba[gsl]),
                        "wall01": np.ascontiguousarray(wall01[gsl]),
                        "ident": ident,
                        "aT": aTh})
    return in_maps


_NC_CACHE = {}


def run_device(hidden, adj, a, **spmd_kwargs):
    if "nc" not in _NC_CACHE:
        _NC_CACHE["nc"] = build_nc()
    nc = _NC_CACHE["nc"]
    in_maps = prep_inputs(hidden, adj, a)
    res = run_bass_kernel_spmd(nc, in_maps, list(range(NCORES)), **spmd_kwargs)
    # res[c]["out"]: [OCTS, 128, OWC]; [g, u*64+i, (gp,t)*129 + c]
    full = np.concatenate([res.results[c]["out"] for c in range(NCORES)],
                          axis=0)
    full = full.astype(np.float32)
    full = full.reshape(B // 8, 2, N, 2, 2, 129)             # [g, u, i, gp, t, c]
    num = full[..., 0:D]
    den = full[..., D:D + 1]
    outq = (num / den).transpose(0, 3, 4, 1, 2, 5)           # [g, gp, t, u, i, d]
    out = np.ascontiguousarray(outq.reshape(B, N, D))
    return out.astype(np.float32), res


def kernel(hidden, adj, a):
    out, _ = run_device(hidden, adj, a)
    return out


# revision 33
# speedup vs baseline: 1.0224x; 1.0224x over previous
"""Bass/Trainium2 kernel for nn_LocalAggregator (GNN message passing).

Math per batch b (hidden [64,128], adj [64,64] in {0..4}, a [4,128]):
    e_k[i,j] = leakyrelu_{0.2}( sum_d hidden[i,d]*hidden[j,d]*a[k,d] )
    alpha    = softmax_j( where(adj==k+1, e_k, -9e15) )
    out      = alpha @ hidden

Device strategy (8 cores, 64 batches/core, OCT = 8 batches/iter):
  - e_k is SYMMETRIC in (i,j): masking with the host-TRANSPOSED
    adjacency yields transposed attention weights directly.
  - w_all[d,(k,g',l,j)] = hT * a_k: HALF shipped from host (k=0,1),
    half computed on device (k=2 on DVE tensor_scalar, k=3 on ACT
    scaled-Copy, both with per-partition a_k scalars) -- trades 2.1MB
    of HBM traffic for ~1us/oct of idle engine time.  The DMA'd half
    lands directly in the same SBUF tile the computed half writes, so
    e-matmul rhs views span one tile.
  - The one-hot additive mask ships as fp8 {0,-192} bytes inside the
    bf16 input tensor (bitcast view on SBUF) and is ADDED into the
    e-PSUM by an fp8 identity matmul (start=True) before the
    e-matmuls accumulate.  The per-k select then becomes a single
    max-reduce; lrelu keeps masked entries <= -38 so exp ~ 0 there.
    ACT per gp: Prelu[128] + Exp[128] only.
  - out-matmuls run ONE OCT BEHIND the e-chain (software pipelining)
    and scratch filler matmuls bridge PE gaps: the PE only reaches
    full clock (2.4GHz) after ~3us of gap-free execution.
  - ones-column in hh makes the out-matmul emit the softmax
    denominator; normalization on HOST.
"""

import numpy as np
import ml_dtypes

from contextlib import ExitStack

import concourse.bass as bass
import concourse.tile as tile
from concourse import bacc, mybir
from concourse._compat import with_exitstack
from concourse.bass_utils import run_bass_kernel_spmd

BF16 = mybir.dt.bfloat16
FP16 = mybir.dt.float16
FP8 = mybir.dt.float8e4
F32 = mybir.dt.float32
ALU = mybir.AluOpType
ACTF = mybir.ActivationFunctionType

B, N, D, K = 512, 64, 128, 4
NCORES = 8
BPC = B // NCORES          # 64 batches per core
OCTS = BPC // 8            # 8 octs of 8 batches per core
HHW = 132                  # hidden cols + ones col + pad
MASK = -192.0              # additive mask; exact in fp8e4m3, exp() -> ~0
CWA = 512 + 528 + 512      # hT8 | hh8 | indm(fp8 as bf16) = 1552 cols
CWB = 1024                 # wall8 k=0,1 half (2048B rows)
OWC = 4 * 129              # out tile cols: (num 128 | denom) x (gp,t)


@with_exitstack
def _kernel_body(ctx, tc, cmba_d, wall_d, id_d, at_d, out_d):
    nc = tc.nc

    const_pool = ctx.enter_context(tc.tile_pool(name="const", bufs=1))
    ina_pool = ctx.enter_context(tc.tile_pool(name="inpa", bufs=6))
    wl_pool = ctx.enter_context(tc.tile_pool(name="wlp", bufs=4))
    work_pool = ctx.enter_context(tc.tile_pool(name="work", bufs=4))
    wal_pool = ctx.enter_context(tc.tile_pool(name="walp", bufs=6))
    psum_pool = ctx.enter_context(tc.tile_pool(name="psum", bufs=3, space="PSUM"))
    opsum_pool = ctx.enter_context(tc.tile_pool(name="opsum", bufs=2, space="PSUM"))
    scr_pool = ctx.enter_context(tc.tile_pool(name="scr", bufs=1, space="PSUM"))
    out_pool = ctx.enter_context(tc.tile_pool(name="outp", bufs=4))

    ident = const_pool.tile([128, 128], FP8, tag="ident")
    nc.sync.dma_start(out=ident[:], in_=id_d)
    aT = const_pool.tile([128, 4], F32, tag="aT")
    nc.sync.dma_start(out=aT[:], in_=at_d)

    # PE keep-warm: the tensor engine only reaches full clock after ~3us
    # of gap-free execution.  A scratch stream with no input deps bridges
    # the startup (DMA latency) and the per-oct gaps so the real matmuls
    # run at full rate.  Results are never read.
    fsrc = const_pool.tile([128, 512], FP8, tag="fsrc")
    nc.gpsimd.memset(fsrc[:], 0)
    scr = scr_pool.tile([128, 512], F32, tag="scr")

    def filler(n):
        for _ in range(n):
            nc.tensor.matmul(scr[:], lhsT=fsrc[:, 0:128], rhs=fsrc[:],
                             start=True, stop=True)

    filler(10)

    def out_block(g, wal, hh8, final=False):
        """out matmuls + evac + output DMA for oct g (runs 1 oct behind
        the e-chain so the PE never waits on the ACT/DVE chain).  The
        final oct evacuates per gp-half on both engines with HWDGE DMA
        halves to shorten the drain chain."""
        ops = opsum_pool.tile([128, 1024], F32, tag="ops")
        osb = out_pool.tile([128, OWC], BF16, tag="osb")
        for gp in range(2):
            for l in range(4):
                t, u = l // 2, l % 2
                nc.tensor.matmul(
                    ops[u * 64:(u + 1) * 64,
                        gp * 512 + t * HHW: gp * 512 + (t + 1) * HHW],
                    lhsT=wal[u * 64:(u + 1) * 64,
                             gp * 128 + t * 64: gp * 128 + (t + 1) * 64],
                    rhs=hh8[u * 64:(u + 1) * 64,
                            gp * 264 + t * HHW: gp * 264 + (t + 1) * HHW],
                    start=True, stop=True,
                    tile_position=(u * 64, u * 64))
            if final:
                # evac this gp-half as soon as its 4 matmuls finish
                srcg = (ops[:, gp * 512:gp * 512 + 264]
                        .rearrange("p (t c) -> p t c", t=2)[:, :, 0:129])
                dstg = (osb[:, gp * 258:(gp + 1) * 258]
                        .rearrange("p (t c) -> p t c", t=2))
                if gp == 0:
                    nc.vector.tensor_copy(dstg, srcg)
                else:
                    nc.scalar.activation(dstg, srcg, ACTF.Copy)
                nc.sync.dma_start(out=out_d[g][:, gp * 258:(gp + 1) * 258],
                                  in_=osb[:, gp * 258:(gp + 1) * 258])
        if not final:
            # compact evac (num|den only), alternating DVE/ACT for balance
            src = ops[:].rearrange("p (g q) -> p g q", g=2)[:, :, 0:264]
            src = src.rearrange("p g (t c) -> p g t c", t=2)[:, :, :, 0:129]
            dst = osb[:].rearrange("p (g t c) -> p g t c", g=2, t=2)
            if g % 2 == 0:
                nc.vector.tensor_copy(dst, src)
            else:
                nc.scalar.activation(dst, src, ACTF.Copy)
            nc.gpsimd.dma_start(out=out_d[g], in_=osb[:])

    prev = None
    for g in range(OCTS):
        # oct loads: A = 0:512 hT8 [d,(g',l,i)] | 512:1040 hh8
        # [(u,j),(g',t,c)] | 1040:1552 fp8 mask bytes [(u,x),(k,g',t,y)]
        # B = wall k=0,1 [d,(k01,g',l,j)] -> first half of the wall tile
        cmba = ina_pool.tile([128, CWA], BF16, tag="cmba")
        nc.sync.dma_start(out=cmba[:], in_=cmba_d[g])
        wall = wl_pool.tile([128, 2048], BF16, tag="wall")
        nc.sync.dma_start(out=wall[:, 0:1024], in_=wall_d[g])

        # lagged out-block first: its inputs are long since ready, so the
        # PE stays busy while this oct's DMA lands
        if prev is not None:
            out_block(*prev)

        hT8 = cmba[:, 0:512]
        hh8 = cmba[:, 512:1040]
        im8v = cmba[:, 1040:CWA].bitcast(FP8).rearrange(
            "p (k g ty) -> p k g ty", k=4, g=2)

        # device-computed wall half: k=2 on DVE, k=3 on ACT (per-partition
        # a_k scalars; both engines have slack under the DMA window).  The
        # final oct's pair is priority-hoisted one iteration so it does not
        # queue behind oct-6's elementwise (its data arrives in time, so no
        # head-of-line blocking on the in-order queues).
        from contextlib import nullcontext
        hoist = tc.high_priority(offset=30) if g == OCTS - 1 else nullcontext()
        with hoist:
            nc.vector.tensor_scalar(wall[:, 1024:1536], hT8, aT[:, 2:3],
                                    None, ALU.mult)
            nc.scalar.activation(wall[:, 1536:2048], hT8, ACTF.Copy,
                                 scale=aT[:, 3:4])
        # wall cols are (k, g', l, j): k-stride 512
        wallv = wall[:].rearrange("p (k g l j) -> p k g l j",
                                  k=4, g=2, l=4)

        z = work_pool.tile([128, 256], FP16, tag="z")
        for gp in range(2):
            # ---- e4[(u,x), (k,t,y)] = e_k[x,y] + mask (1-bank tile) ----
            e4 = psum_pool.tile([128, 512], F32, tag="e4")
            e4v = e4[:].rearrange("p (k t y) -> p k t y", k=4, t=2)
            # mask lands first (identity matmul, start=True resets bank)
            nc.tensor.matmul(
                e4[:].rearrange("p (k ty) -> p k ty", k=4),
                lhsT=ident[:],
                rhs=im8v[:, :, gp, :],
                start=True, stop=False)
            for l in range(4):
                t, u = l // 2, l % 2
                nc.tensor.matmul(
                    e4v[u * 64:(u + 1) * 64, :, t, :],
                    lhsT=hT8[:, gp * 256 + l * 64: gp * 256 + (l + 1) * 64],
                    rhs=wallv[:, :, gp, l, :],
                    start=False, stop=True,
                    tile_position=(0, u * 64))

            # ---- per-k select: ONE max-reduce over the k axis (PSUM) ----
            nc.vector.tensor_reduce(
                z[:, gp * 128:(gp + 1) * 128],
                e4[:].rearrange("p (k ty) -> p ty k", k=4),
                mybir.AxisListType.X, ALU.max)

        if g < OCTS - 1:
            # ---- leakyrelu (masked stays <= -38) then exp, both gps ----
            pz = work_pool.tile([128, 256], FP16, tag="pz")
            nc.scalar.activation(pz[:], z[:], ACTF.Prelu, alpha=0.2)
            wal = wal_pool.tile([128, 256], BF16, tag="wal")
            nc.scalar.activation(wal[:], pz[:], ACTF.Exp)

            # bridge the oct-boundary PE gap to hold the clock at full
            # speed (more in the drain phase, where chain waits downclock)
            filler(1 if g < 5 else 3)
            prev = (g, wal, hh8)
        else:
            # ---- final oct: gp-granular drain.  Each half's exp ->
            # out-matmuls -> evac -> DMA proceeds without waiting for the
            # other half's reduce/activation chain.
            filler(3)
            ops = opsum_pool.tile([128, 1024], F32, tag="ops")
            osb = out_pool.tile([128, OWC], BF16, tag="osb")
            for gp in range(2):
                pzg = work_pool.tile([128, 128], FP16, tag="pzf")
                zg = z[:, gp * 128:(gp + 1) * 128]
                if gp == 0:
                    nc.scalar.activation(pzg[:], zg, ACTF.Prelu, alpha=0.2)
                else:
                    # lrelu = max(0.2*z, z) on DVE: keeps ACT's serial
                    # queue to [prelu0, exp0, exp1] in the drain
                    nc.vector.scalar_tensor_tensor(pzg[:], zg, 0.2, zg,
                                                   ALU.mult, ALU.max)
                walg = wal_pool.tile([128, 128], BF16, tag="walf")
                nc.scalar.activation(walg[:], pzg[:], ACTF.Exp)
                for l in range(4):
                    t, u = l // 2, l % 2
                    nc.tensor.matmul(
                        ops[u * 64:(u + 1) * 64,
                            gp * 512 + t * HHW: gp * 512 + (t + 1) * HHW],
                        lhsT=walg[u * 64:(u + 1) * 64, t * 64:(t + 1) * 64],
                        rhs=hh8[u * 64:(u + 1) * 64,
                                gp * 264 + t * HHW: gp * 264 + (t + 1) * HHW],
                        start=True, stop=True,
                        tile_position=(u * 64, u * 64))
                srcg = (ops[:, gp * 512:gp * 512 + 264]
                        .rearrange("p (t c) -> p t c", t=2)[:, :, 0:129])
                dstg = (osb[:, gp * 258:(gp + 1) * 258]
                        .rearrange("p (t c) -> p t c", t=2))
                if gp == 0:
                    nc.vector.tensor_copy(dstg, srcg)
                else:
                    nc.scalar.activation(dstg, srcg, ACTF.Copy)
                nc.sync.dma_start(out=out_d[g][:, gp * 258:(gp + 1) * 258],
                                  in_=osb[:, gp * 258:(gp + 1) * 258])
                if gp == 0:
                    # bridge the exp1-wait PE gap so gp1's out-matmuls
                    # run at full clock instead of a reset DVFS ramp
                    filler(1)
            prev = None

    if prev is not None:
        out_block(*prev, final=True)


def build_nc():
    nc = bacc.Bacc("TRN2", target_bir_lowering=False, debug=False)
    cmba_d = nc.dram_tensor("cmba", [OCTS, 128, CWA], BF16,
                            kind="ExternalInput").ap()
    wall_d = nc.dram_tensor("wall01", [OCTS, 128, CWB], BF16,
                            kind="ExternalInput").ap()
    id_d = nc.dram_tensor("ident", [128, 128], FP8,
                          kind="ExternalInput").ap()
    at_d = nc.dram_tensor("aT", [128, 4], F32,
                          kind="ExternalInput").ap()
    out_d = nc.dram_tensor("out", [OCTS, 128, OWC], BF16,
                           kind="ExternalOutput").ap()
    with tile.TileContext(nc) as tc:
        _kernel_body(tc, cmba_d, wall_d, id_d, at_d, out_d)
    nc.compile()
    return nc


def _octify(x):
    """[B//4, 128, W] -> [B//8, 128, 2*W] pairing consecutive quads."""
    q, p, w = x.shape
    return (x.reshape(q // 2, 2, p, w).transpose(0, 2, 1, 3)
            .reshape(q // 2, p, 2 * w))


def prep_inputs(hidden, adj, a):
    """Host-side packing: bf16/fp8 casts, fused transposed layouts."""
    bf = ml_dtypes.bfloat16
    f8 = ml_dtypes.float8_e4m3
    hidden = np.asarray(hidden, dtype=np.float32)
    adj = np.asarray(adj)
    a = np.asarray(a, dtype=np.float32)

    hb = hidden.astype(bf)                                   # [B, 64, 128]

    # hT_q[q, d, l*64+i] = hidden[4q+l, i, d]
    hTf = (hidden.transpose(0, 2, 1)
           .reshape(B // 4, 4, D, N)
           .transpose(0, 2, 1, 3)
           .reshape(B // 4, D, 4 * N))
    hT = hTf.astype(bf)

    # wall8[oct, d, k*512+g'*256+l*64+j] = hidden[oct*8+g'*4+l, j, d]*a[k,d]
    # shipped half from f32 hidden (single rounding; the device half pays
    # one extra bf16 rounding through hT)
    wallq = (hTf[:, None, :, :]
             * a[None, :, :, None]).astype(bf)               # [q, k, d, (l,j)]
    wall8 = (wallq.reshape(B // 8, 2, K, D, 4 * N)
             .transpose(0, 3, 2, 1, 4)
             .reshape(B // 8, D, 2 * K * 4 * N))             # [oct,d,(k,g,l,j)]
    wall01 = np.ascontiguousarray(wall8[:, :, 0:1024])

    # hh_q[q, u*64+j, t*HHW + c] : hidden rows + ones col for batch 4q+2t+u
    hh = np.zeros((B, N, HHW), dtype=bf)
    hh[:, :, 0:D] = hb
    hh[:, :, D] = bf(1.0)
    hhq = (hh.reshape(B // 4, 2, 2, N, HHW)
           .transpose(0, 2, 3, 1, 4)
           .reshape(B // 4, 2 * N, 2 * HHW))

    # indm[oct, u*64+x, k*256+g'*128+t*64+y] = 0 if adj[b][y,x]==k+1 else MASK
    # with b = oct*8 + g'*4 + t*2 + u; shipped as raw fp8 bytes inside cmba
    adjT = adj.transpose(0, 2, 1)                            # [b, x, y]
    mk = np.where(
        adjT[:, None, :, :] == np.arange(1, 5)[None, :, None, None],
        np.float32(0.0), np.float32(MASK)).astype(f8)        # [b, k, x, y]
    mk = mk.reshape(B // 8, 2, 2, 2, K, N, N)                # [o,g',t,u,k,x,y]
    indm = np.ascontiguousarray(
        mk.transpose(0, 3, 5, 4, 1, 2, 6).reshape(B // 8, 128, 1024))

    cmba = np.concatenate(
        [np.ascontiguousarray(_octify(hT)).view(np.uint8),
         np.ascontiguousarray(_octify(hhq)).view(np.uint8),
         indm.view(np.uint8)], axis=2).view(bf)
    cmba = np.ascontiguousarray(cmba)                        # [B//8, 128, CWA]

    ident = np.ascontiguousarray(np.eye(128, dtype=f8))
    aTh = np.ascontiguousarray(a.T.astype(np.float32))       # [128, 4]

    in_maps = []
    for c in range(NCORES):
        gsl = slice(c * OCTS, (c + 1) * OCTS)
        in_maps.append({"cmba": np.ascontiguousarray(cmba[gsl]),
                        "wall01": np.ascontiguousarray(wall01[gsl]),
                        "ident": ident,
                        "aT": aTh})
    return in_maps


_NC_CACHE = {}


def run_device(hidden, adj, a, **spmd_kwargs):
    if "nc" not in _NC_CACHE:
        _NC_CACHE["nc"] = build_nc()
    nc = _NC_CACHE["nc"]
    in_maps = prep_inputs(hidden, adj, a)
    res = run_bass_kernel_spmd(nc, in_maps, list(range(NCORES)), **spmd_kwargs)
    # res[c]["out"]: [OCTS, 128, OWC]; [g, u*64+i, (gp,t)*129 + c]
    full = np.concatenate([res.results[c]["out"] for c in range(NCORES)],
                          axis=0)
    full = full.astype(np.float32)
    full = full.reshape(B // 8, 2, N, 2, 2, 129)             # [g, u, i, gp, t, c]
    num = full[..., 0:D]
    den = full[..., D:D + 1]
    outq = (num / den).transpose(0, 3, 4, 1, 2, 5)           # [g, gp, t, u, i, d]
    out = np.ascontiguousarray(outq.reshape(B, N, D))
    return out.astype(np.float32), res


def kernel(hidden, adj, a):
    out, _ = run_device(hidden, adj, a)
    return out
